# revision 22
# baseline (speedup 1.0000x reference)
"""Trainium2 Bass kernel for nn_EntInit (gnn_message_passing).

feat[n, :] = mean over incoming edges e (dst[e] == n) of T[etypes[e], :]
where T = concat(rel_head_emb, rel_tail_emb)  (row etype of T directly).

Key identity: with Hist[n, r] = #{edges e : dst[e]==n, etypes[e]==r},
    feat[n, :] = (Hist[n, :] @ T) / max(count[n], 1),  count = Hist.sum(1).
So the whole op is one-hot matmuls — no per-edge gather/scatter DMA.

Sharding: nodes are split into 8 contiguous ranges of 6250 (one per core);
edges are routed to the core owning their dst (host-side counting sort).
Per core, nodes are processed in 49 blocks of 128; each block's edges are
packed into TPB fixed 128-edge tiles (padded with etype=400 -> zero row).

Device, per 128-edge tile:
    OneHot[e, r] = (etype[e] == r)   r in [0,512)   (DVE is_equal, fp16 in)
    Mem[e, n]    = (dst_rel[e] == n) n in [0,128)
    HistT_c     += OneHot[:, c*128:...]^T @ Mem     (PE, PSUM accumulate)
Per 128-node block (after its TPB tiles):
    sums[n, 0:257] = sum_c HistT_c^T @ wt_c         (wt = [hi|lo|count] bf16)
    feat[n]        = (sums[:,0:128]+sums[:,128:256]) / max(count, 1)
One 64KB DMA per block writes feat rows straight to HBM. bf16 hi/lo split
of T gives ~f32 precision; Hist counts <=56 are bf16-exact.
"""
import sys

sys.path.insert(0, "/opt/trn_rl_repo")

import numpy as np
import ml_dtypes

import concourse.bass as bass
import concourse.bacc as bacc
import concourse.mybir as mybir
import concourse.tile as tile
from concourse.masks import make_identity

NUM_REL = 200
N_TYPES = 2 * NUM_REL          # 400 real relation rows
PAD_TYPE = N_TYPES             # type 400: zero row for padding edges
N_CORES = 8
P = 128
N_NODES = 50000
NPC = N_NODES // N_CORES       # 6250 nodes per core
NB = -(-NPC // P)              # 49 node blocks per core
RT = 512                       # one-hot width (>= 401, 4 chunks of 128)
WCOL = 257                     # wt cols: 128 hi | 128 lo | 1 count
BF16 = ml_dtypes.bfloat16

_prog_cache: dict = {}


def _build_program(tpb: int):
    """One SPMD program; cores differ only in input data."""
    import os as _os
    kvar = _os.environ.get("KVAR", "full")
    t_all = NB * tpb
    nc = bacc.Bacc("TRN2", debug=False, num_devices=1)
    etc_d = nc.dram_tensor("etc", [P, t_all], mybir.dt.float16,
                           kind="ExternalInput").ap()
    drc_d = nc.dram_tensor("drc", [P, t_all], mybir.dt.float16,
                           kind="ExternalInput").ap()
    wt_d = nc.dram_tensor("wt", [P, 4 * WCOL], mybir.dt.bfloat16,
                          kind="ExternalInput").ap()
    cst_d = nc.dram_tensor("cst", [P, RT + P], mybir.dt.float16,
                           kind="ExternalInput").ap()
    feat = nc.dram_tensor("feat", [NB * P, P], mybir.dt.float32,
                          kind="ExternalOutput").ap()

    with tile.TileContext(nc) as tc:
        with (
            tc.tile_pool(name="const", bufs=1) as const_tp,
            tc.tile_pool(name="oh", bufs=3) as oh_tp,
            tc.tile_pool(name="mem", bufs=3) as mem_tp,
            tc.tile_pool(name="blk", bufs=2) as blk_tp,
            tc.tile_pool(name="hist", bufs=2, space="PSUM") as hist_tp,
            tc.tile_pool(name="tr", bufs=2, space="PSUM") as tr_tp,
            tc.tile_pool(name="sums", bufs=2, space="PSUM") as sums_tp,
        ):
            etc = const_tp.tile([P, t_all], mybir.dt.float16)
            drc = const_tp.tile([P, t_all], mybir.dt.float16)
            wt = const_tp.tile([P, 4 * WCOL], mybir.dt.bfloat16)
            cst = const_tp.tile([P, RT + P], mybir.dt.float16)
            ident = const_tp.tile([P, P], mybir.dt.bfloat16)
            make_identity(nc, ident[:])
            nc.sync.dma_start(out=etc[:], in_=etc_d[:])
            nc.sync.dma_start(out=drc[:], in_=drc_d[:])
            nc.sync.dma_start(out=wt[:], in_=wt_d[:])
            nc.sync.dma_start(out=cst[:], in_=cst_d[:])
            iota_r = cst[:, 0:RT]        # row r-index, same on every partition
            iota_n = cst[:, RT:RT + P]   # row n-index

            for b in range(NB):
                # Batched one-hot build: ONE DVE is_equal per block for all
                # tpb tiles (3D dual-broadcast), instead of one per tile.
                oh = oh_tp.tile([P, tpb, RT], mybir.dt.bfloat16, tag="oh")
                mem = mem_tp.tile([P, tpb, P], mybir.dt.bfloat16, tag="mem")
                if kvar == "nodve":
                    nc.vector.memset(oh[:, 0, 0:1], 0.0)
                    nc.vector.memset(mem[:, 0, 0:1], 0.0)
                else:
                    nc.vector.tensor_tensor(
                        out=oh[:],
                        in0=etc[:, b * tpb:(b + 1) * tpb].unsqueeze(2)
                            .to_broadcast([P, tpb, RT]),
                        in1=iota_r.unsqueeze(1).to_broadcast([P, tpb, RT]),
                        op=mybir.AluOpType.is_equal,
                    )
                    nc.vector.tensor_tensor(
                        out=mem[:],
                        in0=drc[:, b * tpb:(b + 1) * tpb].unsqueeze(2)
                            .to_broadcast([P, tpb, P]),
                        in1=iota_n.unsqueeze(1).to_broadcast([P, tpb, P]),
                        op=mybir.AluOpType.is_equal,
                    )
                # hist[n, r] accumulated over the block's tiles in ONE PSUM
                # accumulation group (interleaved groups in one bank lose
                # read-modify-writes on TRN2).
                hist = hist_tp.tile([P, RT], mybir.dt.float32, tag="hist")
                n_mm = 1 if kvar == "nomm" else tpb
                for t in range(n_mm):
                    nc.tensor.matmul(
                        out=hist[:], lhsT=mem[:, t, :], rhs=oh[:, t, :],
                        start=(t == 0), stop=(t == n_mm - 1),
                    )

                if kvar == "noblk":
                    histb0 = blk_tp.tile([P, RT], mybir.dt.bfloat16, tag="histb")
                    nc.scalar.copy(out=histb0[:, 0:1], in_=hist[:, 0:1])
                    if b == NB - 1:
                        ftz = blk_tp.tile([P, P], mybir.dt.float32, tag="ft")
                        nc.vector.memset(ftz[:], 0.0)
                        for bb in range(NB):
                            nc.sync.dma_start(
                                out=feat[bb * P:(bb + 1) * P, :], in_=ftz[:])
                    continue
                histb = blk_tp.tile([P, RT], mybir.dt.bfloat16, tag="histb")
                nc.scalar.copy(out=histb[:], in_=hist[:])
                # 4 self-contained transposes into one PSUM tile, one copy out
                trp = tr_tp.tile([P, RT], mybir.dt.bfloat16, tag="tr")
                for c in range(4):
                    nc.tensor.transpose(
                        out=trp[:, c * P:(c + 1) * P],
                        in_=histb[:, c * P:(c + 1) * P],
                        identity=ident[:],
                    )
                histT = blk_tp.tile([P, RT], mybir.dt.bfloat16, tag="histT")
                nc.scalar.copy(out=histT[:], in_=trp[:])
                sums = sums_tp.tile([P, WCOL], mybir.dt.float32, tag="sums")
                for c in range(4):
                    nc.tensor.matmul(
                        out=sums[:], lhsT=histT[:, c * P:(c + 1) * P],
                        rhs=wt[:, c * WCOL:(c + 1) * WCOL],
                        start=(c == 0), stop=(c == 3),
                    )
                ssb = blk_tp.tile([P, WCOL], mybir.dt.float32, tag="ssb")
                nc.scalar.copy(out=ssb[:], in_=sums[:])
                cm = blk_tp.tile([P, 1], mybir.dt.float32, tag="cm")
                nc.vector.tensor_scalar_max(out=cm[:], in0=ssb[:, 256:257],
                                            scalar1=1.0)
                rc = blk_tp.tile([P, 1], mybir.dt.float32, tag="rc")
                nc.vector.reciprocal(out=rc[:], in_=cm[:])
                hs = blk_tp.tile([P, P], mybir.dt.float32, tag="hs")
                nc.vector.tensor_add(out=hs[:], in0=ssb[:, 0:P],
                                     in1=ssb[:, P:2 * P])
                ft = blk_tp.tile([P, P], mybir.dt.float32, tag="ft")
                nc.vector.tensor_scalar_mul(out=ft[:], in0=hs[:], scalar1=rc[:])
                nc.sync.dma_start(out=feat[b * P:(b + 1) * P, :], in_=ft[:])

    nc.compile()
    return nc


def _host_prepare(et: np.ndarray, d: np.ndarray):
    """Counting-sort edges into (core, block, tile-slot) layout. Returns
    per-core input maps (etc/drc, [128, NB*tpb] fp16) and tpb."""
    E = et.shape[0]
    core = d // NPC
    within = d - core * NPC
    blk = within >> 7
    rel = within & 127
    bg = core * NB + blk                       # global block id, monotone in d
    counts = np.bincount(bg, minlength=N_CORES * NB)
    tpb = max(1, int(-(-counts.max() // P)))
    cap = tpb * P

    order = np.argsort(bg, kind="stable")
    starts = np.concatenate(([0], np.cumsum(counts)[:-1]))
    pos = np.arange(E, dtype=np.int64) - starts[bg[order]]
    slot = bg[order] * cap + pos

    ET = np.full(N_CORES * NB * cap, float(PAD_TYPE), np.float16)
    DR = np.zeros(N_CORES * NB * cap, np.float16)
    ET[slot] = et[order].astype(np.float16)
    DR[slot] = rel[order].astype(np.float16)

    ET = ET.reshape(N_CORES, NB * tpb, P)
    DR = DR.reshape(N_CORES, NB * tpb, P)
    in_maps = []
    for k in range(N_CORES):
        in_maps.append({
            "etc": np.ascontiguousarray(ET[k].T),
            "drc": np.ascontiguousarray(DR[k].T),
        })
    return in_maps, tpb


def _make_wt(head: np.ndarray, tail: np.ndarray) -> np.ndarray:
    """[128, 4*257] bf16: chunk c holds rows 128c..128c+127 of
    [hi(128) | lo(128) | count(1)]; rows >= 400 are zero."""
    W = np.zeros((4 * P, WCOL), np.float32)
    full = np.concatenate([head, tail], axis=0).astype(np.float32)  # [400,128]
    hi = full.astype(BF16).astype(np.float32)
    W[:N_TYPES, 0:P] = hi
    W[:N_TYPES, P:2 * P] = full - hi
    W[:N_TYPES, 256] = 1.0
    return np.ascontiguousarray(
        W.reshape(4, P, WCOL).transpose(1, 0, 2).reshape(P, 4 * WCOL)
    ).astype(BF16)


def _make_cst() -> np.ndarray:
    c = np.zeros((P, RT + P), np.float16)
    c[:, 0:RT] = np.arange(RT, dtype=np.float16)[None, :]
    c[:, RT:RT + P] = np.arange(P, dtype=np.float16)[None, :]
    return c


_runner_cache: dict = {}


def _get_runner(nc):
    """Cached jitted SPMD executor. Output buffers must be jit parameters
    (the neuronx-cc hook requires bass_exec operands == outer jit params),
    so they are donated; a separate jitted zeros-maker creates them on
    device, avoiding any host->device zero upload."""
    key = id(nc)
    if key in _runner_cache:
        return _runner_cache[key]
    import jax
    import jax.numpy as jnp
    from jax.experimental.shard_map import shard_map
    from jax.sharding import Mesh, PartitionSpec
    from concourse import bass2jax
    from concourse.bass2jax import _bass_exec_p, partition_id_tensor

    bass2jax.install_neuronx_cc_hook()

    in_names, out_names, out_avals = [], [], []
    for alloc in nc.m.functions[0].allocations:
        if not isinstance(alloc, mybir.MemoryLocationSet):
            continue
        name = alloc.memorylocations[0].name
        if alloc.kind == "ExternalInput":
            if nc.partition_id_tensor is None or name != nc.partition_id_tensor.name:
                in_names.append(name)
        elif alloc.kind == "ExternalOutput":
            shape = tuple(alloc.tensor_shape)
            dtype = mybir.dt.np(alloc.dtype)
            out_names.append(name)
            out_avals.append(jax.core.ShapedArray(shape, dtype))
    n_params = len(in_names)
    all_names = list(in_names) + list(out_names)
    if nc.partition_id_tensor is not None:
        all_names.append(nc.partition_id_tensor.name)

    def _body(*args):
        operands = list(args)
        if nc.partition_id_tensor is not None:
            operands.append(partition_id_tensor())
        outs = _bass_exec_p.bind(
            *operands,
            out_avals=tuple(out_avals),
            in_names=tuple(all_names),
            out_names=tuple(out_names),
            lowering_input_output_aliases=(),
            sim_require_finite=True,
            sim_require_nnan=True,
            nc=nc,
        )
        return tuple(outs)

    devices = jax.devices()[:N_CORES]
    mesh = Mesh(np.asarray(devices), ("core",))
    in_specs = (PartitionSpec("core"),) * (n_params + len(out_names))
    out_specs = (PartitionSpec("core"),) * len(out_names)
    # No donation: the kernel fully overwrites every output row, so the
    # "output" operands (hook requires them as jit params) can be ONE cached
    # device-resident buffer reused across calls — one dispatch per call.
    fn = jax.jit(
        shard_map(_body, mesh=mesh, in_specs=in_specs, out_specs=out_specs,
                  check_rep=False),
        keep_unused=True,
    )

    def _zbody():
        return tuple(jnp.zeros(av.shape, av.dtype) for av in out_avals)

    zfn = jax.jit(shard_map(_zbody, mesh=mesh, in_specs=(),
                            out_specs=(PartitionSpec("core"),) * len(out_names),
                            check_rep=False))
    r = (fn, zfn, in_names, out_names, out_avals)
    _runner_cache[key] = r
    return r


_zeros_cache: dict = {}


def _run_spmd_cached(nc, in_maps):
    fn, zfn, in_names, out_names, out_avals = _get_runner(nc)
    concat_in = [np.concatenate([m[n] for m in in_maps], axis=0)
                 for n in in_names]
    key = id(nc)
    if key not in _zeros_cache:
        import jax
        zs = zfn()
        jax.block_until_ready(zs)
        _zeros_cache[key] = zs
    out_arrs = fn(*concat_in, *_zeros_cache[key])
    return {
        name: np.asarray(out_arrs[i]).reshape(N_CORES, *out_avals[i].shape)
        for i, name in enumerate(out_names)
    }


def kernel(etypes, dst, rel_head_emb, rel_tail_emb, n_nodes):
    et = np.asarray(etypes).astype(np.int64)
    d = np.asarray(dst).astype(np.int64)
    head = np.asarray(rel_head_emb, dtype=np.float32)
    tail = np.asarray(rel_tail_emb, dtype=np.float32)
    nn = int(n_nodes)
    assert nn == N_NODES, f"kernel compiled for {N_NODES} nodes, got {nn}"

    in_maps, tpb = _host_prepare(et, d)
    wt = _make_wt(head, tail)
    cst = _make_cst()
    for m in in_maps:
        m["wt"] = wt
        m["cst"] = cst

    import os as _os
    _key = (tpb, _os.environ.get("KVAR", "full"))
    if _key not in _prog_cache:
        _prog_cache[_key] = _build_program(tpb)
    nc = _prog_cache[_key]

    import time as _time
    _t0 = _time.perf_counter()
    res = _run_spmd_cached(nc, in_maps)
    global LAST_DEVICE_WALL
    LAST_DEVICE_WALL = _time.perf_counter() - _t0

    feat = res["feat"]                          # [8, NB*128, 128]
    return np.ascontiguousarray(
        feat[:, :NPC, :].reshape(N_NODES, P)).astype(np.float32)
